# revision 1
# baseline (speedup 1.0000x reference)
"""MoE layer (dense top-3-of-8 gating) on 8 Trainium2 NeuronCores.

Strategy: data-parallel over the batch. Each core processes B/8 = 1024
tokens with the full expert weights (bf16, host-pre-tiled). Gating
(softmax + top-3 mask) runs in fp32 on-device; expert FFNs run in bf16
with fp32 PSUM accumulation; the weighted combine is a fused DVE
scalar_tensor_tensor (out += w[:,e] * y_e) per expert.

Self-contained: hardcodes shapes B=8192, D=1024, H=4096, E=8, K=3.
"""

import os
import sys
from contextlib import ExitStack

sys.path.insert(0, "/opt/trn_rl_repo")

import ml_dtypes
import numpy as np

import concourse.bass as bass
import concourse.tile as tile
from concourse import bacc, mybir

P = 128
F32 = mybir.dt.float32
BF16 = mybir.dt.bfloat16
AF = mybir.ActivationFunctionType
ALU = mybir.AluOpType
AX = mybir.AxisListType


def build_moe(nc, TOKC, D, H, E=8, K=3, has_b2=False):
    """Emit the per-core Tile program. All SBUF tiles are 2D [128, free]."""
    ND, NH, TT = D // P, H // P, TOKC // P
    TB = min(512, TOKC)
    NTB = TOKC // TB
    NS = TB // P
    NC = min(512, D)
    NJ = D // NC
    NH2 = NH + (1 if has_b2 else 0)

    xt32_d = nc.declare_dram_parameter("xt32", [ND, P, TOKC], F32, isOutput=False)
    wgx_d = nc.declare_dram_parameter("wgx", [P, ND * E], F32, isOutput=False)
    w1x_d = nc.declare_dram_parameter("w1x", [E, NH, P, ND * P], BF16, isOutput=False)
    w2x_d = nc.declare_dram_parameter("w2x", [E, NH2, P, D], BF16, isOutput=False)
    b1x_d = nc.declare_dram_parameter("b1x", [P, E * NH], F32, isOutput=False)
    out_d = nc.declare_dram_parameter("out", [TT, P, D], F32, isOutput=True)

    with ExitStack() as ctx:
        tc = ctx.enter_context(tile.TileContext(nc))
        const = ctx.enter_context(tc.tile_pool(name="const", bufs=1))
        accp = ctx.enter_context(tc.tile_pool(name="acc", bufs=1))
        xtp = ctx.enter_context(tc.tile_pool(name="xtp", bufs=1))
        hp = ctx.enter_context(tc.tile_pool(name="hp", bufs=1))
        w1p = ctx.enter_context(tc.tile_pool(name="w1p", bufs=4))
        w2p = ctx.enter_context(tc.tile_pool(name="w2p", bufs=NH2 + 3))
        smallp = ctx.enter_context(tc.tile_pool(name="smallp", bufs=4))
        xt32p = ctx.enter_context(tc.tile_pool(name="xt32p", bufs=2))

        wg_sb = const.tile([P, ND * E], F32, tag="wg")
        nc.sync.dma_start(wg_sb, wgx_d[:])
        b1_sb = const.tile([P, E * NH], F32, tag="b1")
        nc.sync.dma_start(b1_sb, b1x_d[:])
        w_sb = const.tile([P, TT * E], F32, tag="w")

        out_acc = accp.tile([P, TT * D], F32, tag="acc")
        nc.vector.memset(out_acc, 0.0)
        xt_sb = xtp.tile([P, ND * TOKC], BF16, tag="xt")

        # ---- gating: logits = x @ Wg^T in fp32; one PSUM bank per token tile ----
        with tc.tile_pool(name="pgp", bufs=TT, space="PSUM") as pgp:
            pgs = [pgp.tile([P, E], F32, tag="pg", name=f"pg_{t}") for t in range(TT)]
            for d in range(ND):
                xt32t = xt32p.tile([P, TOKC], F32, tag="xt32")
                nc.sync.dma_start(xt32t, xt32_d[d])
                nc.vector.tensor_copy(xt_sb[:, d * TOKC : (d + 1) * TOKC], xt32t)
                for t in range(TT):
                    nc.tensor.matmul(
                        pgs[t],
                        lhsT=xt32t[:, t * P : (t + 1) * P],
                        rhs=wg_sb[:, d * E : (d + 1) * E],
                        start=(d == 0),
                        stop=(d == ND - 1),
                    )
            # softmax over E then keep top-K values (no renormalization)
            for t in range(TT):
                pg = pgs[t]
                negm = smallp.tile([P, 1], F32, tag="negm")
                nc.vector.tensor_reduce(negm, pg, axis=AX.X, op=ALU.max, negate=True)
                ge = smallp.tile([P, E], F32, tag="ge")
                nc.scalar.activation(ge, pg, AF.Exp, bias=negm, scale=1.0)
                ssum = smallp.tile([P, 1], F32, tag="ssum")
                nc.vector.reduce_sum(ssum, ge, axis=AX.X)
                rsum = smallp.tile([P, 1], F32, tag="rsum")
                nc.vector.reciprocal(rsum, ssum)
                gn = smallp.tile([P, E], F32, tag="gn")
                nc.vector.tensor_scalar(gn, ge, rsum, None, op0=ALU.mult)
                mx8 = smallp.tile([P, 8], F32, tag="mx8")
                nc.vector.max(mx8, gn)
                msk = smallp.tile([P, E], F32, tag="msk")
                nc.vector.tensor_scalar(msk, gn, mx8[:, K - 1 : K], None, op0=ALU.is_ge)
                nc.vector.tensor_mul(w_sb[:, t * E : (t + 1) * E], gn, msk)

        # ---- experts ----
        php = ctx.enter_context(tc.tile_pool(name="php", bufs=3, space="PSUM"))
        pyp = ctx.enter_context(tc.tile_pool(name="pyp", bufs=3, space="PSUM"))
        for e in range(E):
            w2t = []
            for i in range(NH2):
                w2ti = w2p.tile([P, D], BF16, tag="w2", name=f"w2_{e}_{i}")
                nc.sync.dma_start(w2ti, w2x_d[e, i])
                w2t.append(w2ti)
            for tb in range(NTB):
                hT = hp.tile([P, NH2 * TB], BF16, tag="hT", name=f"h_{e}_{tb}")
                for i in range(NH):
                    w1t = w1p.tile([P, ND * P], BF16, tag="w1", name=f"w1_{e}_{tb}_{i}")
                    nc.sync.dma_start(w1t, w1x_d[e, i])
                    ph = php.tile([P, TB], F32, tag="ph")
                    for d in range(ND):
                        nc.tensor.matmul(
                            ph,
                            lhsT=w1t[:, d * P : (d + 1) * P],
                            rhs=xt_sb[:, d * TOKC + tb * TB : d * TOKC + (tb + 1) * TB],
                            start=(d == 0),
                            stop=(d == ND - 1),
                        )
                    # h^T = relu(ph + b1) -> bf16
                    nc.scalar.activation(
                        hT[:, i * TB : (i + 1) * TB],
                        ph,
                        AF.Relu,
                        bias=b1_sb[:, e * NH + i : e * NH + i + 1],
                        scale=1.0,
                    )
                if has_b2:
                    # augmented H row of ones -> adds b2 via the extra W2 tile
                    nc.vector.memset(hT[:, NH * TB : (NH + 1) * TB], 0.0)
                    nc.vector.memset(hT[0:1, NH * TB : (NH + 1) * TB], 1.0)
                for s in range(NS):
                    t = tb * NS + s
                    for j in range(NJ):
                        py = pyp.tile([P, NC], F32, tag="py")
                        for i in range(NH2):
                            nc.tensor.matmul(
                                py,
                                lhsT=hT[:, i * TB + s * P : i * TB + (s + 1) * P],
                                rhs=w2t[i][:, j * NC : (j + 1) * NC],
                                start=(i == 0),
                                stop=(i == NH2 - 1),
                            )
                        oslice = out_acc[:, t * D + j * NC : t * D + (j + 1) * NC]
                        nc.vector.scalar_tensor_tensor(
                            out=oslice,
                            in0=py,
                            scalar=w_sb[:, t * E + e : t * E + e + 1],
                            in1=oslice,
                            op0=ALU.mult,
                            op1=ALU.add,
                        )
        for t in range(TT):
            nc.sync.dma_start(out_d[t], out_acc[:, t * D : (t + 1) * D])
    return nc


def build_moe_sparse(nc, TOKC, D, H, E=8, K=3, has_b2=False, CAPB=256, BLKTOK=512):
    """Sparse (routed) MoE: per 512-token block, compact the tokens routed to
    each expert into <=CAPB slots via one-hot permutation matmuls, run the
    FFN on CAP=NBLK*CAPB slots/expert, scatter-add back with the transposed
    permutation, scaling by the fp32 gate weight per token partition.

    Requires per-(block, expert) routed counts <= CAPB (checked on host;
    caller falls back to the dense builder otherwise).
    """
    ND, NH, TT = D // P, H // P, TOKC // P
    NBLK = TOKC // BLKTOK
    NTL = BLKTOK // P  # token tiles per block
    NST = CAPB // P  # slot tiles per block
    NC = min(512, D)
    NJ = D // NC
    NH2 = NH + (1 if has_b2 else 0)

    xt32_d = nc.declare_dram_parameter("xt32", [ND, P, TOKC], F32, isOutput=False)
    xnat_d = nc.declare_dram_parameter("xnat", [TT, P, D], BF16, isOutput=False)
    wgx_d = nc.declare_dram_parameter("wgx", [P, ND * E], F32, isOutput=False)
    w1x_d = nc.declare_dram_parameter("w1x", [E, NH, P, ND * P], BF16, isOutput=False)
    w2x_d = nc.declare_dram_parameter("w2x", [E, NH2, P, D], BF16, isOutput=False)
    b1x_d = nc.declare_dram_parameter("b1x", [P, E * NH], F32, isOutput=False)
    iota_d = nc.declare_dram_parameter("iotax", [P, CAPB], F32, isOutput=False)
    idbf_d = nc.declare_dram_parameter("identbf", [P, P], BF16, isOutput=False)
    id32_d = nc.declare_dram_parameter("ident32", [P, P], F32, isOutput=False)
    out_d = nc.declare_dram_parameter("out", [TT, P, D], F32, isOutput=True)

    with ExitStack() as ctx:
        tc = ctx.enter_context(tile.TileContext(nc))
        const = ctx.enter_context(tc.tile_pool(name="const", bufs=1))
        accp = ctx.enter_context(tc.tile_pool(name="acc", bufs=1))
        xp = ctx.enter_context(tc.tile_pool(name="xp", bufs=1))
        smallp = ctx.enter_context(tc.tile_pool(name="smallp", bufs=4))

        wg_sb = const.tile([P, ND * E], F32, tag="wg")
        nc.sync.dma_start(wg_sb, wgx_d[:])
        b1_sb = const.tile([P, E * NH], F32, tag="b1")
        nc.sync.dma_start(b1_sb, b1x_d[:])
        iota_sb = const.tile([P, CAPB], F32, tag="iota")
        nc.sync.dma_start(iota_sb, iota_d[:])
        idbf_sb = const.tile([P, P], BF16, tag="idbf")
        nc.sync.dma_start(idbf_sb, idbf_d[:])
        id32_sb = const.tile([P, P], F32, tag="id32")
        nc.sync.dma_start(id32_sb, id32_d[:])
        w_sb = const.tile([P, TT * E], F32, tag="w")
        pos_sb = const.tile([P, TT * E], F32, tag="pos")

        out_acc = accp.tile([P, TT * D], F32, tag="acc")
        nc.vector.memset(out_acc, 0.0)
        x_sb = xp.tile([P, TT * D], BF16, tag="xnat")
        for t in range(TT):
            nc.sync.dma_start(x_sb[:, t * D : (t + 1) * D], xnat_d[t])

        # ---- gating (fp32) + compaction metadata, in scoped pools so their
        # SBUF returns to the stack before the expert pools open ----
        with tc.tile_pool(name="xt32p", bufs=2) as xt32p, tc.tile_pool(
            name="metap", bufs=1
        ) as metap:
            with tc.tile_pool(name="pgp", bufs=TT, space="PSUM") as pgp:
                pgs = [
                    pgp.tile([P, E], F32, tag="pg", name=f"pg_{t}") for t in range(TT)
                ]
                for d in range(ND):
                    xt32t = xt32p.tile([P, TOKC], F32, tag="xt32")
                    nc.sync.dma_start(xt32t, xt32_d[d])
                    for t in range(TT):
                        nc.tensor.matmul(
                            pgs[t],
                            lhsT=xt32t[:, t * P : (t + 1) * P],
                            rhs=wg_sb[:, d * E : (d + 1) * E],
                            start=(d == 0),
                            stop=(d == ND - 1),
                        )
                for t in range(TT):
                    pg = pgs[t]
                    negm = smallp.tile([P, 1], F32, tag="negm")
                    nc.vector.tensor_reduce(
                        negm, pg, axis=AX.X, op=ALU.max, negate=True
                    )
                    ge = smallp.tile([P, E], F32, tag="ge")
                    nc.scalar.activation(ge, pg, AF.Exp, bias=negm, scale=1.0)
                    ssum = smallp.tile([P, 1], F32, tag="ssum")
                    nc.vector.reduce_sum(ssum, ge, axis=AX.X)
                    rsum = smallp.tile([P, 1], F32, tag="rsum")
                    nc.vector.reciprocal(rsum, ssum)
                    gn = smallp.tile([P, E], F32, tag="gn")
                    nc.vector.tensor_scalar(gn, ge, rsum, None, op0=ALU.mult)
                    mx8 = smallp.tile([P, 8], F32, tag="mx8")
                    nc.vector.max(mx8, gn)
                    msk = smallp.tile([P, E], F32, tag="msk")
                    nc.vector.tensor_scalar(
                        msk, gn, mx8[:, K - 1 : K], None, op0=ALU.is_ge
                    )
                    nc.vector.tensor_mul(w_sb[:, t * E : (t + 1) * E], gn, msk)

            # compaction metadata (PSUM transpose pool opens after gating closes)
            with tc.tile_pool(name="ptrp0", bufs=2, space="PSUM") as ptrp0:
                # tA: wT then scan then posm (reused in place); tB: mask
                tA = metap.tile([8, TT * P], F32, tag="tA")
                tB = metap.tile([8, TT * P], F32, tag="tB")
                for t in range(TT):
                    trw = ptrp0.tile([8, P], F32, tag="tr", name=f"trw_{t}")
                    nc.tensor.transpose(trw, w_sb[:, t * E : (t + 1) * E], id32_sb)
                    nc.scalar.activation(tA[:, t * P : (t + 1) * P], trw, AF.Copy)
                nc.vector.tensor_scalar(tB, tA, 0.0, None, op0=ALU.is_gt)
                # block-local inclusive prefix-sum of the mask -> slot positions
                for blk in range(NBLK):
                    sl = slice(blk * BLKTOK, (blk + 1) * BLKTOK)
                    nc.vector.tensor_tensor_scan(
                        tA[:, sl],
                        tB[:, sl],
                        tB[:, sl],
                        0.0,
                        op0=ALU.add,
                        op1=ALU.bypass,
                    )
                nc.vector.tensor_mul(tA, tA, tB)
                nc.vector.tensor_scalar_add(tA, tA, -1.0)
                # transpose back: pos_sb[tok_p, t*E+e] = slot of token (or -1)
                for t in range(TT):
                    trp = ptrp0.tile([P, 8], F32, tag="tr", name=f"trp_{t}")
                    nc.tensor.transpose(
                        trp, tA[:, t * P : (t + 1) * P], id32_sb[0:8, 0:8]
                    )
                    nc.scalar.activation(pos_sb[:, t * E : (t + 1) * E], trp, AF.Copy)

        # ---- expert pools (allocated after the gating pools released) ----
        xep = ctx.enter_context(tc.tile_pool(name="xep", bufs=1))
        hp = ctx.enter_context(tc.tile_pool(name="hp", bufs=1))
        yp = ctx.enter_context(tc.tile_pool(name="yp", bufs=1))
        pp = ctx.enter_context(tc.tile_pool(name="ppool", bufs=2 * NBLK * NTL))
        ptp = ctx.enter_context(tc.tile_pool(name="ptpool", bufs=NBLK))
        w1p = ctx.enter_context(tc.tile_pool(name="w1p", bufs=3))
        w2p = ctx.enter_context(tc.tile_pool(name="w2p", bufs=NH2 + 1))
        pA = ctx.enter_context(tc.tile_pool(name="pA", bufs=2, space="PSUM"))
        pB = ctx.enter_context(tc.tile_pool(name="pB", bufs=2, space="PSUM"))
        pC = ctx.enter_context(tc.tile_pool(name="pC", bufs=4, space="PSUM"))

        # ---- experts ----
        def build_P(e):
            """One-hot routing tiles P[tok_p, slot] for expert e (DVE work,
            emitted one expert ahead so PE never waits on them)."""
            tiles = {}
            for blk in range(NBLK):
                for tl in range(NTL):
                    t = blk * NTL + tl
                    ptile = pp.tile([P, CAPB], BF16, tag="P", name=f"P_{e}_{t}")
                    nc.vector.tensor_scalar(
                        ptile,
                        iota_sb,
                        pos_sb[:, t * E + e : t * E + e + 1],
                        None,
                        op0=ALU.is_equal,
                    )
                    tiles[(blk, tl)] = ptile
            return tiles

        Pnext = build_P(0)
        for e in range(E):
            Pt = Pnext
            ptT = []
            for blk in range(NBLK):
                pt_sb = ptp.tile([P, NST * BLKTOK], BF16, tag="PT", name=f"PT_{e}_{blk}")
                for st in range(NST):
                    for tl in range(NTL):
                        trb = pA.tile([P, P], BF16, tag="pa", name=f"trb_{e}_{blk}_{st}_{tl}")
                        nc.tensor.transpose(
                            trb, Pt[(blk, tl)][:, st * P : (st + 1) * P], idbf_sb
                        )
                        nc.scalar.activation(
                            pt_sb[:, st * BLKTOK + tl * P : st * BLKTOK + (tl + 1) * P],
                            trb,
                            AF.Copy,
                        )
                ptT.append(pt_sb)

            # gather: xTe[d*CAP + blk*CAPB + s] = x^T column of the s-th routed token
            CAP = NBLK * CAPB
            xTe = xep.tile([P, ND * CAP], BF16, tag="xTe", name=f"xTe_{e}")
            for blk in range(NBLK):
                for d in range(ND):
                    pgx = pA.tile([P, CAPB], F32, tag="pa", name="pgx")
                    for tl in range(NTL):
                        t = blk * NTL + tl
                        nc.tensor.matmul(
                            pgx,
                            lhsT=x_sb[:, t * D + d * P : t * D + (d + 1) * P],
                            rhs=Pt[(blk, tl)],
                            start=(tl == 0),
                            stop=(tl == NTL - 1),
                        )
                    xdst = xTe[:, d * CAP + blk * CAPB : d * CAP + (blk + 1) * CAPB]
                    if (blk * ND + d) % 2 == 0:
                        nc.scalar.activation(xdst, pgx, AF.Copy)
                    else:
                        nc.vector.tensor_copy(xdst, pgx)

            w2t = []
            for i in range(NH2):
                w2ti = w2p.tile([P, D], BF16, tag="w2", name=f"w2_{e}_{i}")
                nc.sync.dma_start(w2ti, w2x_d[e, i])
                w2t.append(w2ti)

            # mm1 over all CAP slots in one N=CAP pass; W1 loaded once per (e,i)
            NSTG = CAP // P  # slot tiles across both blocks
            hT = hp.tile([P, NH2 * CAP], BF16, tag="hT", name=f"h_{e}")
            for i in range(NH):
                w1t = w1p.tile([P, ND * P], BF16, tag="w1", name=f"w1_{e}_{i}")
                nc.sync.dma_start(w1t, w1x_d[e, i])
                ph = pA.tile([P, CAP], F32, tag="pa", name="ph")
                for d in range(ND):
                    nc.tensor.matmul(
                        ph,
                        lhsT=w1t[:, d * P : (d + 1) * P],
                        rhs=xTe[:, d * CAP : (d + 1) * CAP],
                        start=(d == 0),
                        stop=(d == ND - 1),
                    )
                nc.scalar.activation(
                    hT[:, i * CAP : (i + 1) * CAP],
                    ph,
                    AF.Relu,
                    bias=b1_sb[:, e * NH + i : e * NH + i + 1],
                    scale=1.0,
                )
            if has_b2:
                nc.vector.memset(hT[:, NH * CAP : (NH + 1) * CAP], 0.0)
                nc.vector.memset(hT[0:1, NH * CAP : (NH + 1) * CAP], 1.0)
            y_sb = yp.tile([P, NSTG * D], BF16, tag="y", name=f"y_{e}")
            for stg in range(NSTG):
                for j in range(NJ):
                    py = pB.tile([P, NC], F32, tag="py", name="py")
                    for i in range(NH2):
                        nc.tensor.matmul(
                            py,
                            lhsT=hT[:, i * CAP + stg * P : i * CAP + (stg + 1) * P],
                            rhs=w2t[i][:, j * NC : (j + 1) * NC],
                            start=(i == 0),
                            stop=(i == NH2 - 1),
                        )
                    nc.scalar.activation(
                        y_sb[:, stg * D + j * NC : stg * D + (j + 1) * NC],
                        py,
                        AF.Copy,
                    )
            if e + 1 < E:
                Pnext = build_P(e + 1)
            # scatter-add back to token rows, scaled by the fp32 gate weight
            for blk in range(NBLK):
                for tl in range(NTL):
                    tg = blk * NTL + tl
                    for j in range(NJ):
                        pc = pC.tile([P, NC], F32, tag="pc", name="pc")
                        for st in range(NST):
                            stg = blk * NST + st
                            nc.tensor.matmul(
                                pc,
                                lhsT=ptT[blk][:, st * BLKTOK + tl * P : st * BLKTOK + (tl + 1) * P],
                                rhs=y_sb[:, stg * D + j * NC : stg * D + (j + 1) * NC],
                                start=(st == 0),
                                stop=(st == NST - 1),
                            )
                        oslice = out_acc[:, tg * D + j * NC : tg * D + (j + 1) * NC]
                        nc.vector.scalar_tensor_tensor(
                            out=oslice,
                            in0=pc,
                            scalar=w_sb[:, tg * E + e : tg * E + e + 1],
                            in1=oslice,
                            op0=ALU.mult,
                            op1=ALU.add,
                        )
        for t in range(TT):
            nc.sync.dma_start(out_d[t], out_acc[:, t * D : (t + 1) * D])
    return nc


def host_routing_max_count(x, Wg, M, BLKTOK, K=3):
    """Max per-(core, block, expert) routed-token count, computed on host."""
    g = x.astype(np.float32) @ Wg.astype(np.float32).T  # [B, E]
    thr = np.sort(g, axis=-1)[:, -K][:, None]
    sel = g >= thr  # [B, E] bool
    B = x.shape[0]
    blocks = sel.reshape(B // BLKTOK, BLKTOK, -1)
    return int(blocks.sum(axis=1).max())


def host_prep(x, Wg, W1, b1, W2, b2, M=8, CAPB=256):
    """Host-side shard + pre-tile. Returns (in_maps, meta)."""
    x = np.ascontiguousarray(np.asarray(x, dtype=np.float32))
    Wg = np.asarray(Wg, dtype=np.float32)
    W1 = np.asarray(W1, dtype=np.float32)
    b1 = np.asarray(b1, dtype=np.float32)
    W2 = np.asarray(W2, dtype=np.float32)
    b2 = np.asarray(b2, dtype=np.float32)

    B, D = x.shape
    E, H, _ = W1.shape
    TOKC = B // M
    TT = TOKC // P
    ND, NH = D // P, H // P
    has_b2 = bool(np.any(b2))
    bf16 = ml_dtypes.bfloat16

    # lhsT tiles for matmul1: w1x[e,i,p,d*P+h] = W1[e, i*P+h, d*P+p]
    w1x = np.ascontiguousarray(
        W1.reshape(E, NH, P, ND, P).transpose(0, 1, 4, 3, 2).reshape(E, NH, P, ND * P)
    ).astype(bf16)
    # rhs tiles for matmul2: w2x[e,i,p,dc] = W2[e, dc, i*P+p]
    w2x = np.ascontiguousarray(W2.transpose(0, 2, 1).reshape(E, NH, P, D)).astype(bf16)
    if has_b2:
        aug = np.zeros((E, 1, P, D), dtype=bf16)
        aug[:, 0, 0, :] = b2.astype(bf16)
        w2x = np.ascontiguousarray(np.concatenate([w2x, aug], axis=1))
    wgx = np.ascontiguousarray(
        Wg.T.reshape(ND, P, E).transpose(1, 0, 2).reshape(P, ND * E)
    )
    b1x = np.ascontiguousarray(b1.reshape(E, NH, P).transpose(2, 0, 1).reshape(P, E * NH))

    iotax = np.ascontiguousarray(
        np.broadcast_to(np.arange(CAPB, dtype=np.float32), (P, CAPB))
    )
    identbf = np.eye(P, dtype=bf16)
    ident32 = np.eye(P, dtype=np.float32)

    in_maps = []
    for c in range(M):
        xc = x[c * TOKC : (c + 1) * TOKC]
        xt32 = np.ascontiguousarray(xc.T.reshape(ND, P, TOKC))
        xnat = np.ascontiguousarray(xc.reshape(TT, P, D)).astype(bf16)
        in_maps.append(
            {
                "xt32": xt32,
                "xnat": xnat,
                "wgx": wgx,
                "w1x": w1x,
                "w2x": w2x,
                "b1x": b1x,
                "iotax": iotax,
                "identbf": identbf,
                "ident32": ident32,
            }
        )
    meta = dict(TOKC=TOKC, D=D, H=H, E=E, has_b2=has_b2)
    return in_maps, meta


def kernel(x, Wg, W1, b1, W2, b2):
    from concourse.bass_utils import run_bass_kernel_spmd

    M = 8
    B, D = np.asarray(x).shape
    TOKC = B // M
    # finest compaction block whose worst-case routed count fits (with margin)
    cfg = None
    for BLKTOK, CAPB in [(256, 128), (512, 256)]:
        if TOKC % BLKTOK == 0 and host_routing_max_count(x, Wg, M, BLKTOK) <= CAPB - 8:
            cfg = (BLKTOK, CAPB)
            break
    in_maps, meta = host_prep(
        x, Wg, W1, b1, W2, b2, M=M, CAPB=cfg[1] if cfg else 256
    )

    nc = bacc.Bacc("TRN2", target_bir_lowering=False, debug=False, num_devices=M)
    if cfg is not None:
        build_moe_sparse(
            nc, meta["TOKC"], D, meta["H"], E=meta["E"], K=3,
            has_b2=meta["has_b2"], CAPB=cfg[1], BLKTOK=cfg[0],
        )
    else:
        build_moe(
            nc, meta["TOKC"], D, meta["H"], E=meta["E"], K=3, has_b2=meta["has_b2"]
        )
    nc.finalize()

    trace = bool(os.environ.get("MOE_TRACE"))
    if trace:
        try:
            import hookshim

            hookshim.install()
        except Exception:
            pass
    res = run_bass_kernel_spmd(nc, in_maps, list(range(M)), trace=trace)
    if trace and res.exec_time_ns is not None:
        print(f"HW exec time: {res.exec_time_ns} ns")

    out = np.concatenate(
        [res.results[c]["out"].reshape(meta["TOKC"], D) for c in range(M)], axis=0
    )
    return out.astype(np.float32)



# revision 3
# speedup vs baseline: 1.8293x; 1.8293x over previous
"""MoE layer (top-3-of-8 gating) on 8 Trainium2 NeuronCores.

Strategy: expert-parallel with host-side routing. The host computes the
gating softmax + top-3 in fp32, gathers each expert's routed tokens into
a compact slot array (NS = max_e ceil(n_e/128)*128 slots, ~1.04x the
ideal load), and pre-tiles the weights. Core c runs expert c's FFN over
its slots: h = relu(x@W1^T + b1) in bf16 with fp32 PSUM accumulation,
y = (h@W2^T) * w_gate fused into the PSUM->SBUF copy. The host combines
with 8 fancy-index adds (and folds in b2 exactly, if nonzero).

Self-contained: hardcodes M=8 cores; shapes B=8192, D=1024, H=4096,
E=8, K=3 come from the inputs.
"""

import os
import sys
from contextlib import ExitStack

sys.path.insert(0, "/opt/trn_rl_repo")

import ml_dtypes
import numpy as np

import concourse.bass as bass
import concourse.tile as tile
from concourse import bacc, mybir

P = 128
F32 = mybir.dt.float32
BF16 = mybir.dt.bfloat16
AF = mybir.ActivationFunctionType
ALU = mybir.AluOpType


def build_expert_ffn(nc, NS, D, H, SC=512):
    """Per-core Tile program: one expert's FFN over NS routed slots.

    DRAM inputs (per-core content, same shapes across cores):
      xt:  [P, ND*NS] bf16  — chunk-major x^T tiles; chunk c at cols
           [ND*c0, ND*(c0+sc)), within it d-major: [d*sc, (d+1)*sc)
      w1:  [NH, P, ND*P] bf16 — w1[i][dp, d*P+hh] = W1[e, i*P+hh, d*P+dp]
      w2:  [NH, P, D] bf16    — w2[i][hp, dc] = W2[e, dc, i*P+hp]
      b1:  [P, NH] f32        — b1[hp, i] = b1[e, i*P+hp]
      wv:  [P, NT] f32        — wv[p, t] = gate weight of slot t*P+p
      out: [NT, P, D] f32     — y[slot, :] scaled by gate weight
    """
    ND, NH, NT = D // P, H // P, NS // P
    chunks = []
    c0 = 0
    while c0 < NS:
        sc = min(SC, NS - c0)
        chunks.append((c0, sc))
        c0 += sc

    xt_d = nc.declare_dram_parameter("xt", [P, ND * NS], BF16, isOutput=False)
    w1_d = nc.declare_dram_parameter("w1", [NH, P, ND * P], BF16, isOutput=False)
    w2_d = nc.declare_dram_parameter("w2", [NH, P, D], BF16, isOutput=False)
    b1_d = nc.declare_dram_parameter("b1", [P, NH], F32, isOutput=False)
    wv_d = nc.declare_dram_parameter("wv", [P, NT], F32, isOutput=False)
    out_d = nc.declare_dram_parameter("out", [NT, P, D], F32, isOutput=True)

    with ExitStack() as ctx:
        tc = ctx.enter_context(tile.TileContext(nc))
        const = ctx.enter_context(tc.tile_pool(name="const", bufs=1))
        w1p = ctx.enter_context(tc.tile_pool(name="w1p", bufs=NH))
        w2p = ctx.enter_context(tc.tile_pool(name="w2p", bufs=NH))
        xtp = ctx.enter_context(tc.tile_pool(name="xtp", bufs=2))
        hp = ctx.enter_context(tc.tile_pool(name="hp", bufs=1))
        outp = ctx.enter_context(tc.tile_pool(name="outp", bufs=3))
        php = ctx.enter_context(tc.tile_pool(name="php", bufs=3, space="PSUM"))
        pyp = ctx.enter_context(tc.tile_pool(name="pyp", bufs=2, space="PSUM"))

        b1_sb = const.tile([P, NH], F32, tag="b1")
        nc.sync.dma_start(b1_sb, b1_d[:])
        wv_sb = const.tile([P, NT], F32, tag="wv")
        nc.sync.dma_start(wv_sb, wv_d[:])
        w1t = []
        w2t = []
        for i in range(NH):
            w1ti = w1p.tile([P, ND * P], BF16, tag="w1", name=f"w1_{i}")
            nc.sync.dma_start(w1ti, w1_d[i])
            w1t.append(w1ti)
            w2ti = w2p.tile([P, D], BF16, tag="w2", name=f"w2_{i}")
            nc.sync.dma_start(w2ti, w2_d[i])
            w2t.append(w2ti)

        hT = hp.tile([P, NH * SC], BF16, tag="hT")
        for c0, sc in chunks:
            xt_sb = xtp.tile([P, ND * SC], BF16, tag="xt", name=f"xt_{c0}")
            nc.sync.dma_start(
                xt_sb[:, : ND * sc], xt_d[:, ND * c0 : ND * (c0 + sc)]
            )
            # mm1: hT[i] = relu(W1 x^T + b1), bf16 out of fp32 PSUM
            for i in range(NH):
                ph = php.tile([P, SC], F32, tag="ph")
                for d in range(ND):
                    nc.tensor.matmul(
                        ph[:, :sc],
                        lhsT=w1t[i][:, d * P : (d + 1) * P],
                        rhs=xt_sb[:, d * sc : (d + 1) * sc],
                        start=(d == 0),
                        stop=(d == ND - 1),
                    )
                nc.scalar.activation(
                    hT[:, i * sc : i * sc + sc],
                    ph[:, :sc],
                    AF.Relu,
                    bias=b1_sb[:, i : i + 1],
                    scale=1.0,
                )
            # mm2: y[t] = (hT^T W2) * w_gate[t]; N=512 chunks of D per PSUM bank
            NC = min(512, D)
            NJ = D // NC
            for s in range(sc // P):
                t = c0 // P + s
                y = outp.tile([P, D], F32, tag="y")
                for j in range(NJ):
                    py = pyp.tile([P, NC], F32, tag="py")
                    for i in range(NH):
                        nc.tensor.matmul(
                            py,
                            lhsT=hT[:, i * sc + s * P : i * sc + (s + 1) * P],
                            rhs=w2t[i][:, j * NC : (j + 1) * NC],
                            start=(i == 0),
                            stop=(i == NH - 1),
                        )
                    nc.scalar.activation(
                        y[:, j * NC : (j + 1) * NC],
                        py,
                        AF.Copy,
                        scale=wv_sb[:, t : t + 1],
                    )
                nc.sync.dma_start(out_d[t], y)
    return nc


def host_route(x, Wg, K=3):
    """Gating softmax + top-K on host, fp32. Returns (w_be, sel_idx)."""
    g = x.astype(np.float32) @ Wg.astype(np.float32).T  # [B, E]
    g = g - g.max(axis=1, keepdims=True)
    eg = np.exp(g)
    gating = eg / eg.sum(axis=1, keepdims=True)  # [B, E] fp32
    # stable descending argsort matches jax.lax.top_k tie-breaking
    idx = np.argsort(-gating, axis=1, kind="stable")[:, :K]  # [B, K]
    w_be = np.zeros_like(gating)
    rows = np.arange(gating.shape[0])[:, None]
    w_be[rows, idx] = gating[rows, idx]
    return w_be, idx


def host_prep(x, W1, b1, W2, w_be, M, SC=512):
    """Per-expert gather + weight pre-tiling. Returns (in_maps, meta)."""
    x = np.asarray(x, dtype=np.float32)
    W1 = np.asarray(W1, dtype=np.float32)
    b1 = np.asarray(b1, dtype=np.float32)
    W2 = np.asarray(W2, dtype=np.float32)
    B, D = x.shape
    E, H, _ = W1.shape
    ND, NH = D // P, H // P
    bf16 = ml_dtypes.bfloat16

    tok_idx = [np.nonzero(w_be[:, e])[0] for e in range(E)]
    n_e = [len(t) for t in tok_idx]
    NS = max(P, -(-max(n_e) // P) * P)
    NT = NS // P

    in_maps = []
    for e in range(E):
        xg = np.zeros((NS, D), dtype=np.float32)
        xg[: n_e[e]] = x[tok_idx[e]]
        # chunk-major x^T: [P, ND*NS], chunk c cols d-major
        blocks = []
        c0 = 0
        while c0 < NS:
            sc = min(SC, NS - c0)
            blk = xg[c0 : c0 + sc].T.reshape(ND, P, sc).transpose(1, 0, 2)
            blocks.append(blk.reshape(P, ND * sc))
            c0 += sc
        xt = np.ascontiguousarray(np.concatenate(blocks, axis=1)).astype(bf16)

        w1x = np.ascontiguousarray(
            W1[e].reshape(NH, P, ND, P).transpose(0, 3, 2, 1).reshape(NH, P, ND * P)
        ).astype(bf16)
        w2x = np.ascontiguousarray(W2[e].T.reshape(NH, P, D)).astype(bf16)
        b1x = np.ascontiguousarray(b1[e].reshape(NH, P).T)

        wsl = np.zeros(NS, dtype=np.float32)
        wsl[: n_e[e]] = w_be[tok_idx[e], e]
        wvx = np.ascontiguousarray(wsl.reshape(NT, P).T)

        in_maps.append({"xt": xt, "w1": w1x, "w2": w2x, "b1": b1x, "wv": wvx})
    meta = dict(NS=NS, D=D, H=H, E=E, tok_idx=tok_idx, n_e=n_e)
    return in_maps, meta


def kernel(x, Wg, W1, b1, W2, b2):
    from concourse.bass_utils import run_bass_kernel_spmd

    M = 8
    x = np.asarray(x)
    B, D = x.shape
    E, H, _ = np.asarray(W1).shape
    assert E == M, (E, M)

    w_be, _ = host_route(x, Wg, K=3)
    in_maps, meta = host_prep(x, W1, b1, W2, w_be, M=M)

    nc = bacc.Bacc("TRN2", target_bir_lowering=False, debug=False, num_devices=M)
    build_expert_ffn(nc, meta["NS"], D, H)
    nc.finalize()

    trace = bool(os.environ.get("MOE_TRACE"))
    if trace:
        try:
            import hookshim

            hookshim.install()
        except Exception:
            pass
    res = run_bass_kernel_spmd(nc, in_maps, list(range(M)), trace=trace)
    if trace and res.exec_time_ns is not None:
        print(f"HW exec time: {res.exec_time_ns} ns")

    out = np.zeros((B, D), dtype=np.float32)
    for e in range(E):
        ye = res.results[e]["out"].reshape(meta["NS"], D)
        out[meta["tok_idx"][e]] += ye[: meta["n_e"][e]]
    b2 = np.asarray(b2, dtype=np.float32)
    if np.any(b2):
        out += w_be @ b2
    return out


# revision 4
# speedup vs baseline: 1.9378x; 1.0593x over previous
"""MoE layer (top-3-of-8 gating) on 8 Trainium2 NeuronCores.

Strategy: expert-parallel with host-side routing. The host computes the
gating softmax + top-3 in fp32, gathers each expert's routed tokens into
a compact slot array (NS = max_e ceil(n_e/128)*128 slots, ~1.04x the
ideal load), and pre-tiles the weights. Core c runs expert c's FFN over
its slots: h = relu(x@W1^T + b1) in bf16 with fp32 PSUM accumulation,
y = (h@W2^T) * w_gate fused into the PSUM->SBUF copy. The host combines
with 8 fancy-index adds (and folds in b2 exactly, if nonzero).

Self-contained: hardcodes M=8 cores; shapes B=8192, D=1024, H=4096,
E=8, K=3 come from the inputs.
"""

import os
import sys
from contextlib import ExitStack

sys.path.insert(0, "/opt/trn_rl_repo")

import ml_dtypes
import numpy as np

import concourse.bass as bass
import concourse.tile as tile
from concourse import bacc, mybir

P = 128
F32 = mybir.dt.float32
BF16 = mybir.dt.bfloat16
AF = mybir.ActivationFunctionType
ALU = mybir.AluOpType


def build_expert_ffn(nc, NS, D, H, SC=512):
    """Per-core Tile program: one expert's FFN over NS routed slots.

    DRAM inputs (per-core content, same shapes across cores):
      xt:  [P, ND*NS] bf16  — chunk-major x^T tiles; chunk c at cols
           [ND*c0, ND*(c0+sc)), within it d-major: [d*sc, (d+1)*sc)
      w1:  [NH, P, ND*P] bf16 — w1[i][dp, d*P+hh] = W1[e, i*P+hh, d*P+dp]
      w2:  [NH, P, D] bf16    — w2[i][hp, dc] = W2[e, dc, i*P+hp]
      b1:  [P, NH] f32        — b1[hp, i] = b1[e, i*P+hp]
      wv:  [P, NT] f32        — wv[p, t] = gate weight of slot t*P+p
      out: [NT, P, D] f32     — y[slot, :] scaled by gate weight
    """
    ND, NH, NT = D // P, H // P, NS // P
    chunks = []
    c0 = 0
    while c0 < NS:
        sc = min(SC, NS - c0)
        chunks.append((c0, sc))
        c0 += sc

    xt_d = nc.declare_dram_parameter("xt", [P, ND * NS], BF16, isOutput=False)
    w1_d = nc.declare_dram_parameter("w1", [NH, P, ND * P], BF16, isOutput=False)
    w2_d = nc.declare_dram_parameter("w2", [NH, P, D], BF16, isOutput=False)
    b1_d = nc.declare_dram_parameter("b1", [P, NH], F32, isOutput=False)
    wv_d = nc.declare_dram_parameter("wv", [P, NT], F32, isOutput=False)
    out_d = nc.declare_dram_parameter("out", [NT, P, D], F32, isOutput=True)

    with ExitStack() as ctx:
        tc = ctx.enter_context(tile.TileContext(nc))
        const = ctx.enter_context(tc.tile_pool(name="const", bufs=1))
        w1p = ctx.enter_context(tc.tile_pool(name="w1p", bufs=NH))
        w2p = ctx.enter_context(tc.tile_pool(name="w2p", bufs=NH))
        xtp = ctx.enter_context(tc.tile_pool(name="xtp", bufs=2))
        hp = ctx.enter_context(tc.tile_pool(name="hp", bufs=1))
        outp = ctx.enter_context(tc.tile_pool(name="outp", bufs=3))
        php = ctx.enter_context(tc.tile_pool(name="php", bufs=3, space="PSUM"))
        pyp = ctx.enter_context(tc.tile_pool(name="pyp", bufs=2, space="PSUM"))

        def load_chunk(c0, sc):
            # split across DMA queues so the load lands in ~sc*ND*2/8 bytes/queue
            xt_sb = xtp.tile([P, ND * SC], BF16, tag="xt", name=f"xt_{c0}")
            for d in range(ND):
                nc.sync.dma_start(
                    xt_sb[:, d * sc : (d + 1) * sc],
                    xt_d[:, ND * c0 + d * sc : ND * c0 + (d + 1) * sc],
                )
            return xt_sb

        # first x chunk ahead of the weight stream: it gates the first matmul
        xt_first = load_chunk(*chunks[0])
        b1_sb = const.tile([P, NH], F32, tag="b1")
        nc.sync.dma_start(b1_sb, b1_d[:])
        wv_sb = const.tile([P, NT], F32, tag="wv")
        nc.sync.dma_start(wv_sb, wv_d[:])
        w1t = []
        w2t = []
        for i in range(NH):
            w1ti = w1p.tile([P, ND * P], BF16, tag="w1", name=f"w1_{i}")
            nc.sync.dma_start(w1ti, w1_d[i])
            w1t.append(w1ti)
        for i in range(NH):
            w2ti = w2p.tile([P, D], BF16, tag="w2", name=f"w2_{i}")
            nc.sync.dma_start(w2ti, w2_d[i])
            w2t.append(w2ti)

        hT = hp.tile([P, NH * SC], BF16, tag="hT")
        for ci, (c0, sc) in enumerate(chunks):
            xt_sb = xt_first if ci == 0 else load_chunk(c0, sc)
            # mm1: hT[i] = relu(W1 x^T + b1), bf16 out of fp32 PSUM
            for i in range(NH):
                ph = php.tile([P, SC], F32, tag="ph")
                for d in range(ND):
                    nc.tensor.matmul(
                        ph[:, :sc],
                        lhsT=w1t[i][:, d * P : (d + 1) * P],
                        rhs=xt_sb[:, d * sc : (d + 1) * sc],
                        start=(d == 0),
                        stop=(d == ND - 1),
                    )
                nc.scalar.activation(
                    hT[:, i * sc : i * sc + sc],
                    ph[:, :sc],
                    AF.Relu,
                    bias=b1_sb[:, i : i + 1],
                    scale=1.0,
                )
            # mm2: y[t] = (hT^T W2) * w_gate[t]; N=512 chunks of D per PSUM bank
            NC = min(512, D)
            NJ = D // NC
            for s in range(sc // P):
                t = c0 // P + s
                y = outp.tile([P, D], F32, tag="y")
                for j in range(NJ):
                    py = pyp.tile([P, NC], F32, tag="py")
                    for i in range(NH):
                        nc.tensor.matmul(
                            py,
                            lhsT=hT[:, i * sc + s * P : i * sc + (s + 1) * P],
                            rhs=w2t[i][:, j * NC : (j + 1) * NC],
                            start=(i == 0),
                            stop=(i == NH - 1),
                        )
                    nc.scalar.activation(
                        y[:, j * NC : (j + 1) * NC],
                        py,
                        AF.Copy,
                        scale=wv_sb[:, t : t + 1],
                    )
                nc.sync.dma_start(out_d[t], y)
    return nc


def host_route(x, Wg, K=3):
    """Gating softmax + top-K on host, fp32. Returns (w_be, sel_idx)."""
    g = x.astype(np.float32) @ Wg.astype(np.float32).T  # [B, E]
    g = g - g.max(axis=1, keepdims=True)
    eg = np.exp(g)
    gating = eg / eg.sum(axis=1, keepdims=True)  # [B, E] fp32
    # stable descending argsort matches jax.lax.top_k tie-breaking
    idx = np.argsort(-gating, axis=1, kind="stable")[:, :K]  # [B, K]
    w_be = np.zeros_like(gating)
    rows = np.arange(gating.shape[0])[:, None]
    w_be[rows, idx] = gating[rows, idx]
    return w_be, idx


def host_prep(x, W1, b1, W2, w_be, M, SC=512):
    """Per-expert gather + weight pre-tiling. Returns (in_maps, meta)."""
    x = np.asarray(x, dtype=np.float32)
    W1 = np.asarray(W1, dtype=np.float32)
    b1 = np.asarray(b1, dtype=np.float32)
    W2 = np.asarray(W2, dtype=np.float32)
    B, D = x.shape
    E, H, _ = W1.shape
    ND, NH = D // P, H // P
    bf16 = ml_dtypes.bfloat16

    tok_idx = [np.nonzero(w_be[:, e])[0] for e in range(E)]
    n_e = [len(t) for t in tok_idx]
    NS = max(P, -(-max(n_e) // P) * P)
    NT = NS // P

    in_maps = []
    for e in range(E):
        xg = np.zeros((NS, D), dtype=np.float32)
        xg[: n_e[e]] = x[tok_idx[e]]
        # chunk-major x^T: [P, ND*NS], chunk c cols d-major
        blocks = []
        c0 = 0
        while c0 < NS:
            sc = min(SC, NS - c0)
            blk = xg[c0 : c0 + sc].T.reshape(ND, P, sc).transpose(1, 0, 2)
            blocks.append(blk.reshape(P, ND * sc))
            c0 += sc
        xt = np.ascontiguousarray(np.concatenate(blocks, axis=1)).astype(bf16)

        w1x = np.ascontiguousarray(
            W1[e].reshape(NH, P, ND, P).transpose(0, 3, 2, 1).reshape(NH, P, ND * P)
        ).astype(bf16)
        w2x = np.ascontiguousarray(W2[e].T.reshape(NH, P, D)).astype(bf16)
        b1x = np.ascontiguousarray(b1[e].reshape(NH, P).T)

        wsl = np.zeros(NS, dtype=np.float32)
        wsl[: n_e[e]] = w_be[tok_idx[e], e]
        wvx = np.ascontiguousarray(wsl.reshape(NT, P).T)

        in_maps.append({"xt": xt, "w1": w1x, "w2": w2x, "b1": b1x, "wv": wvx})
    meta = dict(NS=NS, D=D, H=H, E=E, tok_idx=tok_idx, n_e=n_e)
    return in_maps, meta


def kernel(x, Wg, W1, b1, W2, b2):
    from concourse.bass_utils import run_bass_kernel_spmd

    M = 8
    x = np.asarray(x)
    B, D = x.shape
    E, H, _ = np.asarray(W1).shape
    assert E == M, (E, M)

    w_be, _ = host_route(x, Wg, K=3)
    in_maps, meta = host_prep(x, W1, b1, W2, w_be, M=M)

    nc = bacc.Bacc("TRN2", target_bir_lowering=False, debug=False, num_devices=M)
    build_expert_ffn(nc, meta["NS"], D, H)
    nc.finalize()

    trace = bool(os.environ.get("MOE_TRACE"))
    if trace:
        try:
            import hookshim

            hookshim.install()
        except Exception:
            pass
    res = run_bass_kernel_spmd(nc, in_maps, list(range(M)), trace=trace)
    if trace and res.exec_time_ns is not None:
        print(f"HW exec time: {res.exec_time_ns} ns")

    out = np.zeros((B, D), dtype=np.float32)
    for e in range(E):
        ye = res.results[e]["out"].reshape(meta["NS"], D)
        out[meta["tok_idx"][e]] += ye[: meta["n_e"][e]]
    b2 = np.asarray(b2, dtype=np.float32)
    if np.any(b2):
        out += w_be @ b2
    return out
